# revision 1
# baseline (speedup 1.0000x reference)
"""Trainium2 Bass kernel: BertCL mean-pool + NT-Xent contrastive loss.

Contract: kernel(last_hidden_states [256,512,768] f32, input_mask [256,512] f32)
-> scalar f32 loss, numerically matching the jax reference.

Strategy (8 NeuronCores, SPMD):
  Batch axis sharded STRIDED: core c owns logical batches {c, c+8, c+16, ...}
  (local j <-> logical c + 8j), so the all-gather of locals [0,16) delivers
  logical batches 0..127 (a full half of z) while locals [16,32) still stream.

  stage 1 (memory-bound): per local batch, stream [512,768] through SBUF as a
    [128, 4*768] tile and reduce the sequence axis with ones-vector matmuls
    accumulating in PSUM -> [1,768] sums staged into one SBUF row.
  Per half (16 batches): AllGather the raw sums (the reference's division by
    the mask row-sum is a per-row positive scalar that cancels exactly in the
    L2 normalization, so it is skipped); after the gather each core
    L2-normalizes the [128,768] half (with 1/tau folded into the norm),
    transposes it via PE into zT, and accumulates the one logits block that
    is ever used, S[0:64, half] = z[0:64] @ z_half.T. The collective SENDs
    are emitted mid-loop (gpsimd stream is otherwise empty); all consume
    work is emitted after the loop so collective latency never stalls the
    in-order engine streams during stage 1. The first half's gather +
    processing hides under the second half's DMA streaming; only the second
    (small, latency-bound) collective plus a ~10us chain is exposed.
  Finish: diag-masked logsumexp over rows 0..63 (exp without max-subtraction
    is safe: logits are cosines/tau in [-2,2]), strict-upper-triangle pair
    sum, final scale -> scalar.

  Measured (paired K-differential, see perf_lab.py): ~198us steady-state,
  ~175us est. single-shot vs the 140.6us per-core HBM roofline; relative
  error vs the fp32 jax reference: 4.4e-7 on hardware.

  NOTE: fused DVE ops (tensor_tensor_reduce, scalar_tensor_tensor) pass
  CoreSim but hang/crash this hardware - only plain DVE ops are used.
"""

import sys
from contextlib import ExitStack

import numpy as np

_REPO = "/opt/trn_rl_repo"
if _REPO not in sys.path:
    sys.path.insert(0, _REPO)

import concourse.bass as bass  # noqa: E402  (kept for callers/debugging)
import concourse.tile as tile  # noqa: E402
from concourse import bacc, bass_utils, mybir  # noqa: E402

N_CORES = 8
B, S, H = 256, 512, 768
B_SH = B // N_CORES  # 32 local batches per core
HALF = B_SH // 2  # 16
N_PAIR = B // 4  # 64
TAU = 0.5
F32 = mybir.dt.float32
AX = mybir.AxisListType
AF = mybir.ActivationFunctionType
NEG = -30000.0  # diagonal mask value; exp(NEG + logit) == 0 exactly in fp32


def _body(
    tc,
    x,
    ident,
    dmask,
    triu,
    cnt,
    out,
    use_collective=True,
    stages=("s1", "cc", "s2"),
):
    nc = tc.nc

    with ExitStack() as ctx:
        const = ctx.enter_context(tc.tile_pool(name="const", bufs=1))
        ones_col = const.tile([128, 1], F32)
        nc.vector.memset(ones_col[:], 1.0)
        idt = const.tile([128, 128], F32)
        nc.sync.dma_start(idt[:], ident[:])

        dram = ctx.enter_context(tc.tile_pool(name="dram", bufs=1, space="DRAM"))
        cc_in = dram.tile([B_SH, H], F32)
        shared = "Shared" if use_collective else "Local"
        # asymmetric split: gather locals [0,24) early (hides under the last 8
        # batches' streaming), locals [24,32) at the end (only 64 logical rows
        # of consume work left after the final latency-bound collective)
        SEG = [(0, 16), (16, 32)]
        cc_o = [
            dram.tile([8 * (j1 - j0), H], F32, addr_space=shared, name=f"cc_o{h}")
            for h, (j0, j1) in enumerate(SEG)
        ]

        # staging row for pooled sums: [1, 32*768] on partition 0
        pooled_sb = const.tile([1, B_SH * H], F32)

        xin = ctx.enter_context(tc.tile_pool(name="xin", bufs=6))
        ps1 = ctx.enter_context(tc.tile_pool(name="ps1", bufs=2, space="PSUM"))
        s2 = ctx.enter_context(tc.tile_pool(name="s2", bufs=1))
        s2t = ctx.enter_context(tc.tile_pool(name="s2t", bufs=2))
        psT = ctx.enter_context(tc.tile_pool(name="psT", bufs=2, space="PSUM"))
        psS = ctx.enter_context(tc.tile_pool(name="psS", bufs=1, space="PSUM"))

        # zT[:, k*256 + p] = z[p, k*128 + q] for partition q (h on partitions)
        zT = s2.tile([128, 6 * B], F32)
        pS = psS.tile([N_PAIR, B], F32)

        def send_half(h):
            """Gather raw sums for local rows [16h,16h+16).

            The reference divides pooled sums by the mask row-sum before
            L2-normalizing; that per-row positive scalar cancels exactly in
            the normalization, so we gather raw sums and normalize the
            gathered rows (same result to ~1ulp, and the pre-collective
            tail shrinks to a single DMA)."""
            j0, j1 = SEG[h]
            nc.sync.dma_start(
                cc_in[j0:j1, :],
                pooled_sb[0:1, j0 * H : j1 * H].rearrange("o (b e) -> o b e", e=H),
            )

            if use_collective:
                nc.gpsimd.collective_compute(
                    "AllGather",
                    mybir.AluOpType.bypass,
                    replica_groups=[list(range(N_CORES))],
                    ins=[cc_in[j0:j1, :].opt()],
                    outs=[cc_o[h].opt()],
                )
            else:
                n = j1 - j0
                for c in range(N_CORES):
                    nc.sync.dma_start(
                        cc_o[h][c * n : (c + 1) * n, :], cc_in[j0:j1, :]
                    )

        def consume_block(h, ja, jb, name):
            """Normalize logical rows [8*ja, 8*jb) from gather h; fill zT cols.

            Gathered row (c, j - SEG[h][0]) holds logical batch c + 8j; the
            permuted 3-D AP (j, c, e) lands partitions in logical order."""
            P = 8 * (jb - ja)  # rows in this block
            col = 8 * ja  # zT column base = first logical row
            zh = s2.tile([P, H], F32, tag=name, name=name)
            src = cc_o[h].rearrange("(c j) e -> j c e", c=N_CORES)
            nc.sync.dma_start(zh[:], src[ja - SEG[h][0] : jb - SEG[h][0]])
            sqs = s2t.tile([P, H], F32, tag=f"sqs{name}", name=f"sqs{name}")
            ssn = s2t.tile([P, 1], F32, tag=f"ssn{name}", name=f"ssn{name}")
            nc.vector.tensor_mul(sqs[:], zh[:], zh[:])
            nc.vector.reduce_sum(out=ssn[:], in_=sqs[:], axis=AX.X)
            # sqrt(TAU * ss): scales z by 1/sqrt(tau) so S = z'z'^T = logits
            nrm = s2t.tile([P, 1], F32, tag=f"nrm{name}", name=f"nrm{name}")
            nc.scalar.activation(nrm[:], ssn[:], AF.Sqrt, scale=TAU)
            rn = s2t.tile([P, 1], F32, tag=f"rn{name}", name=f"rn{name}")
            nc.vector.reciprocal(rn[:], nrm[:])
            nc.vector.tensor_scalar_mul(zh[:], zh[:], rn[:, 0:1])
            for k in range(6):
                pt = psT.tile([128, 128], F32, tag="pt")
                nc.tensor.transpose(
                    pt[:, 0:P], zh[:, k * 128 : (k + 1) * 128], idt[0:P, 0:P]
                )
                nc.vector.tensor_copy(
                    zT[:, k * B + col : k * B + col + P], pt[:, 0:P]
                )

        def logits_block(col, n):
            """S[0:64, col:col+n] += sum_k zT_k[:, 0:64].T @ zT_k[:, col:col+n]"""
            for k in range(6):
                nc.tensor.matmul(
                    pS[:, col : col + n],
                    lhsT=zT[:, k * B : k * B + N_PAIR],
                    rhs=zT[:, k * B + col : k * B + col + n],
                    start=(k == 0),
                    stop=(k == 5),
                )

        # ---- stage 1: per-batch sum over the sequence axis -------------------
        x4 = x.rearrange("b (c p) e -> b p c e", p=128)  # [32, 128, 4, 768]
        for b in range(B_SH):
            if "s1" in stages:
                xt = xin.tile([128, 4 * H], F32)
                nc.sync.dma_start(xt[:], x4[b])
                ps = ps1.tile([1, H], F32)
                for c in range(4):
                    nc.tensor.matmul(
                        ps[:, 0:512],
                        lhsT=ones_col[:, 0:1],
                        rhs=xt[:, c * H : c * H + 512],
                        start=(c == 0),
                        stop=(c == 3),
                    )
                for c in range(4):
                    nc.tensor.matmul(
                        ps[:, 512:H],
                        lhsT=ones_col[:, 0:1],
                        rhs=xt[:, c * H + 512 : (c + 1) * H],
                        start=(c == 0),
                        stop=(c == 3),
                    )
                nc.scalar.copy(pooled_sb[0:1, b * H : (b + 1) * H], ps[:])
            if "cc" in stages:
                if b == SEG[0][1] - 1:
                    send_half(0)
                elif b == SEG[1][1] - 1:
                    send_half(1)

        if "cc" not in stages or "s2" not in stages:
            return
        # each gather carries a full 128-row half of z
        consume_block(0, 0, 16, "zb0")
        logits_block(0, 128)
        consume_block(1, 16, 32, "zb1")
        logits_block(128, 128)

        # ---- finish: masked logsumexp + pair sum ----------------------------
        # pS already holds logits (1/tau folded into the normalization)
        dm = s2.tile([N_PAIR, B], F32)
        nc.sync.dma_start(dm[:], dmask[:])
        sd = s2.tile([N_PAIR, B], F32)
        nc.vector.tensor_add(sd[:], pS[:], dm[:])

        # logits are cosine/tau in [-2,2]: exp is safe without max-subtraction
        et = s2.tile([N_PAIR, B], F32)
        se = s2.tile([N_PAIR, 1], F32)
        nc.scalar.activation(et[:], sd[:], AF.Exp, scale=1.0, accum_out=se[:])
        ld = s2.tile([N_PAIR, 1], F32)
        nc.scalar.activation(ld[:], se[:], AF.Ln)  # logden

        # sum_{i<j<n} (logden[i] - logits[i,j])
        #   = sum_i cnt[i]*logden[i] - sum_ij triu[i,j]*logits[i,j]
        tri_t = s2.tile([N_PAIR, N_PAIR], F32)
        nc.sync.dma_start(tri_t[:], triu[:])
        cnt_t = s2.tile([N_PAIR, 1], F32)
        nc.sync.dma_start(cnt_t[:], cnt[:])
        mt2 = s2.tile([N_PAIR, N_PAIR], F32)
        nc.vector.tensor_mul(mt2[:], sd[0:N_PAIR, 0:N_PAIR], tri_t[:])
        rs = s2.tile([N_PAIR, 1], F32)
        nc.vector.reduce_sum(out=rs[:], in_=mt2[:], axis=AX.X)
        t1 = s2.tile([N_PAIR, 1], F32)
        nc.vector.tensor_mul(t1[:], ld[:], cnt_t[:])
        pr = s2.tile([N_PAIR, 1], F32)
        nc.vector.tensor_sub(pr[:], t1[:], rs[:])

        ptot = psS.tile([1, 1], F32, tag="ptot")
        nc.tensor.matmul(
            ptot[:], lhsT=pr[:], rhs=ones_col[0:N_PAIR, 0:1], start=True, stop=True
        )
        res = s2.tile([1, 1], F32)
        nc.vector.tensor_scalar_mul(res[:], ptot[:], -2.0 / N_PAIR * (N_PAIR - 1))
        nc.sync.dma_start(out[0:1, 0:1], res[:])


def build_nc():
    nc = bacc.Bacc("TRN2", target_bir_lowering=False, debug=False, num_devices=N_CORES)
    x = nc.dram_tensor("x", [B_SH, S, H], F32, kind="ExternalInput")
    ident = nc.dram_tensor("ident", [128, 128], F32, kind="ExternalInput")
    dmask = nc.dram_tensor("dmask", [N_PAIR, B], F32, kind="ExternalInput")
    triu = nc.dram_tensor("triu", [N_PAIR, N_PAIR], F32, kind="ExternalInput")
    cnt = nc.dram_tensor("cnt", [N_PAIR, 1], F32, kind="ExternalInput")
    out = nc.dram_tensor("loss", [1, 1], F32, kind="ExternalOutput")
    with tile.TileContext(nc) as tc:
        _body(
            tc,
            x.ap(),
            ident.ap(),
            dmask.ap(),
            triu.ap(),
            cnt.ap(),
            out.ap(),
        )
    nc.compile()
    return nc


def const_inputs():
    ident = np.eye(128, dtype=np.float32)
    dmask = np.zeros((N_PAIR, B), dtype=np.float32)
    dmask[np.arange(N_PAIR), np.arange(N_PAIR)] = NEG
    triu = np.triu(np.ones((N_PAIR, N_PAIR), dtype=np.float32), k=1)
    cnt = (N_PAIR - 1 - np.arange(N_PAIR, dtype=np.float32)).reshape(N_PAIR, 1)
    return {"ident": ident, "dmask": dmask, "triu": triu, "cnt": cnt}


def make_in_maps(last_hidden_states, input_mask):
    del input_mask  # cancels exactly in the L2 normalization (see half_tail)
    x = np.asarray(last_hidden_states, dtype=np.float32)
    consts = const_inputs()
    return [
        {"x": np.ascontiguousarray(x[c::N_CORES]), **consts}  # logical c+8j
        for c in range(N_CORES)
    ]


_CACHE = {}


def get_nc():
    if "nc" not in _CACHE:
        _CACHE["nc"] = build_nc()
    return _CACHE["nc"]


def kernel(last_hidden_states, input_mask):
    nc = get_nc()
    in_maps = make_in_maps(last_hidden_states, input_mask)
    res = bass_utils.run_bass_kernel_spmd(nc, in_maps, core_ids=list(range(N_CORES)))
    return np.asarray(res.results[0]["loss"], dtype=np.float32).reshape(())



# revision 6
# speedup vs baseline: 1.5206x; 1.5206x over previous
"""Trainium2 Bass kernel: BertCL mean-pool + NT-Xent contrastive loss.

Contract: kernel(last_hidden_states [256,512,768] f32, input_mask [256,512] f32)
-> scalar f32 loss, numerically matching the jax reference.

Strategy (8 NeuronCores, SPMD):
  Batch axis sharded STRIDED: core c owns logical batches {c, c+8, c+16, ...}
  (local j <-> logical c + 8j), so the all-gather of locals [0,16) delivers
  logical batches 0..127 (a full half of z) while locals [16,32) still stream.

  stage 1 (memory-bound): per local batch, stream [512,768] through SBUF as a
    [128, 4*768] tile and reduce the sequence axis with ones-vector matmuls
    accumulating in PSUM -> [1,768] sums staged into one SBUF row.
  Per half (16 batches): AllGather the raw sums (the reference's division by
    the mask row-sum is a per-row positive scalar that cancels exactly in the
    L2 normalization, so it is skipped); after the gather each core
    L2-normalizes the [128,768] half (with 1/tau folded into the norm),
    transposes it via PE into zT, and accumulates the one logits block that
    is ever used, S[0:64, half] = z[0:64] @ z_half.T. The collective SENDs
    are emitted mid-loop (gpsimd stream is otherwise empty); all consume
    work is emitted after the loop so collective latency never stalls the
    in-order engine streams during stage 1. The first half's gather +
    processing hides under the second half's DMA streaming; only the second
    (small, latency-bound) collective plus a ~10us chain is exposed.
  Finish: diag-masked logsumexp over rows 0..63 (exp without max-subtraction
    is safe: logits are cosines/tau in [-2,2]), strict-upper-triangle pair
    sum, final scale -> scalar.

  Measured (paired K-differential, see perf_lab.py): ~198us steady-state,
  ~175us est. single-shot vs the 140.6us per-core HBM roofline; relative
  error vs the fp32 jax reference: 4.4e-7 on hardware.

  NOTE: fused DVE ops (tensor_tensor_reduce, scalar_tensor_tensor) pass
  CoreSim but hang/crash this hardware - only plain DVE ops are used.
"""

import sys
from contextlib import ExitStack

import numpy as np

_REPO = "/opt/trn_rl_repo"
if _REPO not in sys.path:
    sys.path.insert(0, _REPO)

import concourse.bass as bass  # noqa: E402  (kept for callers/debugging)
import concourse.tile as tile  # noqa: E402
from concourse import bacc, bass_utils, mybir  # noqa: E402

N_CORES = 8
B, S, H = 256, 512, 768
B_SH = B // N_CORES  # 32 local batches per core
HALF = B_SH // 2  # 16
N_PAIR = B // 4  # 64
TAU = 0.5
F32 = mybir.dt.float32
F32R = mybir.dt.float32r  # PE fast-fp32 mode: 1 cycle/row at >=256-wide out
X_DT = F32R  # dtype of the streamed input (np binding is float32 either way)
AX = mybir.AxisListType
AF = mybir.ActivationFunctionType
NEG = -30000.0  # diagonal mask value; exp(NEG + logit) == 0 exactly in fp32


def _body(
    tc,
    x,
    ident,
    dmask,
    triu,
    cnt,
    out,
    use_collective=True,
    stages=("s1", "cc", "s2"),
):
    nc = tc.nc

    with ExitStack() as ctx:
        const = ctx.enter_context(tc.tile_pool(name="const", bufs=1))
        ones_col = const.tile([128, 1], F32)
        nc.vector.memset(ones_col[:], 1.0)
        idt = const.tile([128, 128], F32)
        nc.sync.dma_start(idt[:], ident[:])

        dram = ctx.enter_context(tc.tile_pool(name="dram", bufs=1, space="DRAM"))
        cc_in = dram.tile([B_SH, H], F32)
        shared = "Shared" if use_collective else "Local"
        # asymmetric split: gather locals [0,24) early (hides under the last 8
        # batches' streaming), locals [24,32) at the end (only 64 logical rows
        # of consume work left after the final latency-bound collective)
        SEG = [(0, 16), (16, 32)]
        cc_o = [
            dram.tile([8 * (j1 - j0), H], F32, addr_space=shared, name=f"cc_o{h}")
            for h, (j0, j1) in enumerate(SEG)
        ]

        # staging row for pooled sums: [1, 32*768] on partition 0
        pooled_sb = const.tile([1, B_SH * H], F32)

        xin = ctx.enter_context(tc.tile_pool(name="xin", bufs=6))
        ps1 = ctx.enter_context(tc.tile_pool(name="ps1", bufs=2, space="PSUM"))
        s2 = ctx.enter_context(tc.tile_pool(name="s2", bufs=1))
        s2t = ctx.enter_context(tc.tile_pool(name="s2t", bufs=2))
        psT = ctx.enter_context(tc.tile_pool(name="psT", bufs=2, space="PSUM"))
        psS = ctx.enter_context(tc.tile_pool(name="psS", bufs=1, space="PSUM"))

        # zT[:, k*256 + p] = z[p, k*128 + q] for partition q (h on partitions)
        zT = s2.tile([128, 6 * B], F32)
        pS = psS.tile([N_PAIR, B], F32)

        def send_half(h):
            """Gather raw sums for local rows [16h,16h+16).

            The reference divides pooled sums by the mask row-sum before
            L2-normalizing; that per-row positive scalar cancels exactly in
            the normalization, so we gather raw sums and normalize the
            gathered rows (same result to ~1ulp, and the pre-collective
            tail shrinks to a single DMA)."""
            j0, j1 = SEG[h]
            nc.sync.dma_start(
                cc_in[j0:j1, :],
                pooled_sb[0:1, j0 * H : j1 * H].rearrange("o (b e) -> o b e", e=H),
            )

            if use_collective:
                nc.gpsimd.collective_compute(
                    "AllGather",
                    mybir.AluOpType.bypass,
                    replica_groups=[list(range(N_CORES))],
                    ins=[cc_in[j0:j1, :].opt()],
                    outs=[cc_o[h].opt()],
                )
            else:
                n = j1 - j0
                for c in range(N_CORES):
                    nc.sync.dma_start(
                        cc_o[h][c * n : (c + 1) * n, :], cc_in[j0:j1, :]
                    )

        def consume_block(h, ja, jb, name):
            """Normalize logical rows [8*ja, 8*jb) from gather h; fill zT cols.

            Gathered row (c, j - SEG[h][0]) holds logical batch c + 8j; the
            permuted 3-D AP (j, c, e) lands partitions in logical order."""
            P = 8 * (jb - ja)  # rows in this block
            col = 8 * ja  # zT column base = first logical row
            zh = s2.tile([P, H], F32, tag=name, name=name)
            src = cc_o[h].rearrange("(c j) e -> j c e", c=N_CORES)
            nc.sync.dma_start(zh[:], src[ja - SEG[h][0] : jb - SEG[h][0]])
            sqs = s2t.tile([P, H], F32, tag=f"sqs{name}", name=f"sqs{name}")
            ssn = s2t.tile([P, 1], F32, tag=f"ssn{name}", name=f"ssn{name}")
            nc.vector.tensor_mul(sqs[:], zh[:], zh[:])
            nc.vector.reduce_sum(out=ssn[:], in_=sqs[:], axis=AX.X)
            # sqrt(TAU * ss): scales z by 1/sqrt(tau) so S = z'z'^T = logits
            nrm = s2t.tile([P, 1], F32, tag=f"nrm{name}", name=f"nrm{name}")
            nc.scalar.activation(nrm[:], ssn[:], AF.Sqrt, scale=TAU)
            rn = s2t.tile([P, 1], F32, tag=f"rn{name}", name=f"rn{name}")
            nc.vector.reciprocal(rn[:], nrm[:])
            nc.vector.tensor_scalar_mul(zh[:], zh[:], rn[:, 0:1])
            for k in range(6):
                pt = psT.tile([128, 128], F32, tag="pt")
                nc.tensor.transpose(
                    pt[:, 0:P], zh[:, k * 128 : (k + 1) * 128], idt[0:P, 0:P]
                )
                nc.vector.tensor_copy(
                    zT[:, k * B + col : k * B + col + P], pt[:, 0:P]
                )

        def logits_block(col, n):
            """S[0:64, col:col+n] += sum_k zT_k[:, 0:64].T @ zT_k[:, col:col+n]"""
            for k in range(6):
                nc.tensor.matmul(
                    pS[:, col : col + n],
                    lhsT=zT[:, k * B : k * B + N_PAIR],
                    rhs=zT[:, k * B + col : k * B + col + n],
                    start=(k == 0),
                    stop=(k == 5),
                )

        # ---- stage 1: per-batch sum over the sequence axis -------------------
        x4 = x.rearrange("b (c p) e -> b p c e", p=128)  # [32, 128, 4, 768]
        for b in range(B_SH):
            if "s1" in stages:
                xt = xin.tile([128, 4 * H], F32R)
                nc.sync.dma_start(xt[:], x4[b])
                ps = ps1.tile([1, H], F32)
                for c in range(4):
                    nc.tensor.matmul(
                        ps[:, 0:512],
                        lhsT=ones_col[:, 0:1].bitcast(F32R),
                        rhs=xt[:, c * H : c * H + 512],
                        start=(c == 0),
                        stop=(c == 3),
                    )
                for c in range(4):
                    nc.tensor.matmul(
                        ps[:, 512:H],
                        lhsT=ones_col[:, 0:1].bitcast(F32R),
                        rhs=xt[:, c * H + 512 : (c + 1) * H],
                        start=(c == 0),
                        stop=(c == 3),
                    )
                nc.scalar.copy(pooled_sb[0:1, b * H : (b + 1) * H], ps[:])
            if "cc" in stages:
                if b == SEG[0][1] - 1:
                    send_half(0)
                elif b == SEG[1][1] - 1:
                    send_half(1)

        if "cc" not in stages or "s2" not in stages:
            return
        # each gather carries a full 128-row half of z
        consume_block(0, 0, 16, "zb0")
        logits_block(0, 128)
        consume_block(1, 16, 32, "zb1")
        logits_block(128, 128)

        # ---- finish: masked logsumexp + pair sum ----------------------------
        # pS already holds logits (1/tau folded into the normalization)
        dm = s2.tile([N_PAIR, B], F32)
        nc.sync.dma_start(dm[:], dmask[:])
        sd = s2.tile([N_PAIR, B], F32)
        nc.vector.tensor_add(sd[:], pS[:], dm[:])

        # logits are cosine/tau in [-2,2]: exp is safe without max-subtraction
        et = s2.tile([N_PAIR, B], F32)
        se = s2.tile([N_PAIR, 1], F32)
        nc.scalar.activation(et[:], sd[:], AF.Exp, scale=1.0, accum_out=se[:])
        ld = s2.tile([N_PAIR, 1], F32)
        nc.scalar.activation(ld[:], se[:], AF.Ln)  # logden

        # sum_{i<j<n} (logden[i] - logits[i,j])
        #   = sum_i cnt[i]*logden[i] - sum_ij triu[i,j]*logits[i,j]
        tri_t = s2.tile([N_PAIR, N_PAIR], F32)
        nc.sync.dma_start(tri_t[:], triu[:])
        cnt_t = s2.tile([N_PAIR, 1], F32)
        nc.sync.dma_start(cnt_t[:], cnt[:])
        mt2 = s2.tile([N_PAIR, N_PAIR], F32)
        nc.vector.tensor_mul(mt2[:], sd[0:N_PAIR, 0:N_PAIR], tri_t[:])
        rs = s2.tile([N_PAIR, 1], F32)
        nc.vector.reduce_sum(out=rs[:], in_=mt2[:], axis=AX.X)
        t1 = s2.tile([N_PAIR, 1], F32)
        nc.vector.tensor_mul(t1[:], ld[:], cnt_t[:])
        pr = s2.tile([N_PAIR, 1], F32)
        nc.vector.tensor_sub(pr[:], t1[:], rs[:])

        ptot = psS.tile([1, 1], F32, tag="ptot")
        nc.tensor.matmul(
            ptot[:], lhsT=pr[:], rhs=ones_col[0:N_PAIR, 0:1], start=True, stop=True
        )
        res = s2.tile([1, 1], F32)
        nc.vector.tensor_scalar_mul(res[:], ptot[:], -2.0 / N_PAIR * (N_PAIR - 1))
        nc.sync.dma_start(out[0:1, 0:1], res[:])


def build_nc():
    nc = bacc.Bacc("TRN2", target_bir_lowering=False, debug=False, num_devices=N_CORES)
    x = nc.dram_tensor("x", [B_SH, S, H], X_DT, kind="ExternalInput")
    ident = nc.dram_tensor("ident", [128, 128], F32, kind="ExternalInput")
    dmask = nc.dram_tensor("dmask", [N_PAIR, B], F32, kind="ExternalInput")
    triu = nc.dram_tensor("triu", [N_PAIR, N_PAIR], F32, kind="ExternalInput")
    cnt = nc.dram_tensor("cnt", [N_PAIR, 1], F32, kind="ExternalInput")
    out = nc.dram_tensor("loss", [1, 1], F32, kind="ExternalOutput")
    with tile.TileContext(nc) as tc:
        _body(
            tc,
            x.ap(),
            ident.ap(),
            dmask.ap(),
            triu.ap(),
            cnt.ap(),
            out.ap(),
        )
    nc.compile()
    return nc


def const_inputs():
    ident = np.eye(128, dtype=np.float32)
    dmask = np.zeros((N_PAIR, B), dtype=np.float32)
    dmask[np.arange(N_PAIR), np.arange(N_PAIR)] = NEG
    triu = np.triu(np.ones((N_PAIR, N_PAIR), dtype=np.float32), k=1)
    cnt = (N_PAIR - 1 - np.arange(N_PAIR, dtype=np.float32)).reshape(N_PAIR, 1)
    return {"ident": ident, "dmask": dmask, "triu": triu, "cnt": cnt}


def make_in_maps(last_hidden_states, input_mask):
    del input_mask  # cancels exactly in the L2 normalization (see half_tail)
    x = np.asarray(last_hidden_states, dtype=np.float32)
    consts = const_inputs()
    return [
        {"x": np.ascontiguousarray(x[c::N_CORES]), **consts}  # logical c+8j
        for c in range(N_CORES)
    ]


_CACHE = {}


def get_nc():
    if "nc" not in _CACHE:
        _CACHE["nc"] = build_nc()
    return _CACHE["nc"]


def kernel(last_hidden_states, input_mask):
    nc = get_nc()
    in_maps = make_in_maps(last_hidden_states, input_mask)
    res = bass_utils.run_bass_kernel_spmd(nc, in_maps, core_ids=list(range(N_CORES)))
    return np.asarray(res.results[0]["loss"], dtype=np.float32).reshape(())

